# revision 25
# baseline (speedup 1.0000x reference)
"""Distributed single-head causal attention for Trainium2 (8 NeuronCores).

Problem: x:[4,2048,1024] f32, Wq/Wk/Wv/Wo:[1024,1024], b*:[1024]
  q = x@Wq.T+bq; k = x@Wk.T+bk; v = x@Wv.T+bv
  scores = (q@k.T)/sqrt(1024) causal-masked; out = softmax(scores)@v @Wo.T + bo

Sharding (data-parallel pairs, K AND V exchanged by block parity, strict-SPMD):
  8 cores = 4 batches x 2 cores/batch. The 16 query blocks (128 rows) of a
  batch split by parity: even core takes even blocks, odd core odd blocks.
  Every core runs 8 "slots" with the compile-time schedule T_s = 256*(s+1)
  over keys in LOGICAL order -> identical instruction streams, balanced
  causal work, causality via a host-built tail mask.

K and V are projected only for the core's own parity blocks (from the same
parity-compact x input as Q) and exchanged within the pair by FOUR 1MB
DRAM-bounce AllGathers that chain back-to-back on the CC core: K rows
0-511 (mesh output = logical key blocks 0-7, the g0 scores' needs), K rows
512-1023, V blocks 0-7, V blocks 8-15 (attnV g0/g1's needs respectively).
AllGather output is rank-ordered, so ccout[0] is the even core's half on
BOTH cores; readbacks interleave the halves straight into logical order -
no rank awareness anywhere.

Queue discipline (in-order engine streams + semaphore waits head-of-line
block everything behind them):
  scalar: WK loads -> K ships -> WO -> phase-B exps/copies/stores(g0).
  sync:   XP/WV loads -> ALL readbacks (its phase-B work, the g1 stores,
          is at the very end so the mesh-semaphore waits block nothing).
  gpsimd: small loads/broadcasts, WQ+mask (software DGE, needed late),
          V ships, all four collective triggers.

The attn transpose (ATT -> ATT_T for the attnV matmul) runs on the PE via
identity matmul, NOT dma_start_transpose: the tile framework serializes
XBAR transposes with collectives (shared resource), which would park the
V exchanges behind all of phase B. Transposes are issued one slot behind
the scores so the PE never waits on an exp. Phase B runs BOTH score groups
before attnV g0 so the V mesh latency stays hidden. No max-subtraction in
softmax (scores ~N(0,1), exp overflow-safe); normalization (1/l) folds
into the output projection.

Per-core PE: K-own 65536cyc + V-own 65536 + Q 65536 + scores 73728 +
attnV 73728 + out 65536 + transposes ~18k = 428k cycles (~181us @2.37GHz).
"""

import sys

if "/opt/trn_rl_repo" not in sys.path:
    sys.path.insert(0, "/opt/trn_rl_repo")

import numpy as np
import ml_dtypes

import concourse.bass as bass
import concourse.mybir as mybir
from concourse import bacc
from concourse.bass_utils import run_bass_kernel_spmd
from concourse.tile import TileContext

B, S, D = 4, 2048, 1024
NB = S // 128
NSLOT = 8
EC = D // 128
F32 = mybir.dt.float32
BF16 = mybir.dt.bfloat16
NEG = -1.0e9
GROUPS = [[0, 1], [2, 3], [4, 5], [6, 7]]

_compiled = None


def _slot_T(s):
    return 256 * (s + 1)


def _build():
    nc = bacc.Bacc("TRN2", target_bir_lowering=False, debug=False, num_devices=8)

    # xpT: parity-compact x (the core's own 8 blocks) - feeds Q, K-own, V-own
    xpT = nc.dram_tensor("xpT", [128, EC, 1024], BF16, kind="ExternalInput")
    wqT = nc.dram_tensor("wqT", [128, EC, D], BF16, kind="ExternalInput")
    wkT = nc.dram_tensor("wkT", [128, EC, D], BF16, kind="ExternalInput")
    wvT = nc.dram_tensor("wvT", [128, EC, D], BF16, kind="ExternalInput")
    woT = nc.dram_tensor("woT", [128, EC, D], BF16, kind="ExternalInput")
    bq_d = nc.dram_tensor("bq", [128, EC], F32, kind="ExternalInput")
    bk_d = nc.dram_tensor("bk", [128, EC], F32, kind="ExternalInput")
    bv_d = nc.dram_tensor("bv", [1, D], F32, kind="ExternalInput")
    bo_d = nc.dram_tensor("bo", [1, D], F32, kind="ExternalInput")
    mask_d = nc.dram_tensor("mask", [128, NSLOT, 256], F32, kind="ExternalInput")
    ident_d = nc.dram_tensor("ident", [128, 128], BF16, kind="ExternalInput")
    out_d = nc.dram_tensor("out", [NSLOT * 128, D], F32, kind="ExternalOutput")

    inv = 1.0 / 32.0

    with TileContext(nc) as tc:
        with (
            tc.tile_pool(name="persist", bufs=1) as persist,
            tc.tile_pool(name="small", bufs=1) as small,
            tc.tile_pool(name="dram", bufs=1, space="DRAM") as dram,
        ):
            # KTOWN's last read (the bounce ship) is long done before QT's
            # first write, so they share one SBUF slot via the common tag.
            # th-major: each K half (rows th*512..) is contiguous for its ship.
            KTOWN = persist.tile([128, 2, EC, 512], BF16, tag="QTK", name="KTOWN")
            KT = persist.tile([128, EC, S], BF16, tag="KT")  # logical order
            V = persist.tile([128, NB, D], BF16, tag="V")  # logical order
            WO = persist.tile([128, EC, D], BF16, tag="WO")
            MASK = small.tile([128, NSLOT, 256], F32, tag="MASK")
            IDENT = small.tile([128, 128], BF16, tag="IDENT")
            BQ = small.tile([128, EC], F32, tag="BQ")
            BK = small.tile([128, EC], F32, tag="BK")
            RL = small.tile([128, NSLOT], F32, tag="RL")
            BOF = small.tile([128, D], F32, tag="BOF")

            bnc_k_in = dram.tile([128, 2, EC, 512], BF16, name="bnc_k_in")
            bnc_k_out = dram.tile([2, 128, 2, EC, 512], BF16, name="bnc_k_out")
            bnc_v1_in = dram.tile([128, 4, D], BF16, name="bnc_v1_in")
            bnc_v1_out = dram.tile([2, 128, 4, D], BF16, name="bnc_v1_out")
            bnc_v2_in = dram.tile([128, 4, D], BF16, name="bnc_v2_in")
            bnc_v2_out = dram.tile([2, 128, 4, D], BF16, name="bnc_v2_out")

            # ---- phase A: K-own -> AllGathers #1/#2; V-own -> #3/#4;
            #      Q proj + phase-B scores overlap the exchanges ----
            with (
                tc.tile_pool(name="xin", bufs=1) as xin,
                tc.tile_pool(name="wts", bufs=1) as wts,
                tc.tile_pool(name="vo", bufs=1) as vo_pool,
                # 5 bufs: three PSUM banks stay virgin so phase-B's first
                # scores tile can allocate while the Q wave still runs
                tc.tile_pool(name="pa_psum", bufs=5, space="PSUM") as pa_psum,
            ):
                XP = xin.tile([128, EC, 1024], BF16, tag="XP")
                WQ = wts.tile([128, EC, D], BF16, tag="WQ")
                WK = wts.tile([128, EC, D], BF16, tag="WK")
                WV = wts.tile([128, EC, D], BF16, tag="WV")
                BVF = xin.tile([128, D], F32, tag="BVF")
                VOWN = vo_pool.tile([128, NSLOT, D], BF16, tag="VOWN")

                # small loads + broadcasts on the gpsimd queue
                bv_row = small.tile([1, D], F32, tag="bv_row")
                nc.gpsimd.dma_start(out=bv_row[:, :], in_=bv_d[:, :])
                nc.gpsimd.dma_start(out=BK[:, :], in_=bk_d[:, :])
                bo_row = small.tile([1, D], F32, tag="bo_row")
                nc.gpsimd.dma_start(out=bo_row[:, :], in_=bo_d[:, :])
                nc.gpsimd.dma_start(out=IDENT[:, :], in_=ident_d[:, :])
                bq_raw = small.tile([128, EC], F32, tag="bq_raw")
                nc.gpsimd.dma_start(out=bq_raw[:, :], in_=bq_d[:, :])
                nc.gpsimd.partition_broadcast(BVF[:, :], bv_row[:1, :])
                nc.gpsimd.partition_broadcast(BOF[:, :], bo_row[:1, :])

                # K-own wave consumes (WK[:,dc,0:512], XP[:,dc,0:512]) first:
                # load those 128KB pieces ahead (spread across many DMA
                # rings) so the PE starts early and never starves mid-wave.
                for dc in range(EC):
                    nc.scalar.dma_start(out=WK[:, dc, 0:512], in_=wkT[:, dc, 0:512])
                    nc.sync.dma_start(out=XP[:, dc, 0:512], in_=xpT[:, dc, 0:512])
                for dc in range(EC):
                    nc.scalar.dma_start(
                        out=WK[:, dc, 512:1024], in_=wkT[:, dc, 512:1024]
                    )
                    nc.sync.dma_start(
                        out=XP[:, dc, 512:1024], in_=xpT[:, dc, 512:1024]
                    )
                # WV/WQ/mask on sync behind XP; the scalar ring drains after
                # WK so the K ships (scalar) execute the moment their waves
                # finish, and gpsimd stays clear so the collective triggers
                # (which queue in-order there) fire immediately
                for dc in range(EC):
                    nc.sync.dma_start(out=WV[:, dc, :], in_=wvT[:, dc, :])
                for dc in range(EC):
                    nc.sync.dma_start(out=WQ[:, dc, :], in_=wqT[:, dc, :])
                nc.scalar.mul(BQ[:, :], bq_raw[:, :], inv)
                nc.sync.dma_start(out=MASK[:, :, :], in_=mask_d[:, :, :])

                # K-own: 2 x 512-row strips; ship each strip as a contiguous
                # 1MB DMA the moment its waves complete, one K exchange for
                # both (a single mesh keeps the serial CC chain short).
                for th in range(2):
                    for wv2 in range(2):
                        ec0 = 4 * wv2
                        pss = [
                            pa_psum.tile(
                                [128, 512], F32, tag="pa", name=f"pak{th}_{wv2}_{i}"
                            )
                            for i in range(4)
                        ]
                        for dc in range(EC):
                            for i in range(4):
                                nc.tensor.matmul(
                                    pss[i][:, :],
                                    WK[:, dc, (ec0 + i) * 128 : (ec0 + i + 1) * 128],
                                    XP[:, dc, th * 512 : (th + 1) * 512],
                                    start=(dc == 0),
                                    stop=(dc == EC - 1),
                                )
                        for i in range(4):
                            ec = ec0 + i
                            nc.vector.tensor_scalar(
                                out=KTOWN[:, th, ec, :],
                                in0=pss[i][:, :],
                                scalar1=BK[:, ec : ec + 1],
                                scalar2=None,
                                op0=mybir.AluOpType.add,
                            )
                    nc.scalar.dma_start(
                        out=bnc_k_in[:, th, :, :], in_=KTOWN[:, th, :, :]
                    )
                # ccout[0] = even core's half, ccout[1] = odd's on BOTH
                # cores (AllGather output is rank-ordered)
                nc.gpsimd.collective_compute(
                    "AllGather",
                    mybir.AluOpType.bypass,
                    replica_groups=GROUPS,
                    ins=[bnc_k_in.opt()],
                    outs=[bnc_k_out.opt()],
                )

                # WO now, before phase B claims the scalar stream
                for dc in range(EC):
                    nc.scalar.dma_start(out=WO[:, dc, :], in_=woT[:, dc, :])

                # V-own from the same parity-compact x; rows as partitions.
                # Own blocks 0-3 across the pair = logical V blocks 0-7
                # (attnV g0's needs): exchange them as soon as projected.
                for b in range(NSLOT):
                    vps = [
                        pa_psum.tile([128, 512], F32, tag="pa", name=f"pav{b}_{i}")
                        for i in range(2)
                    ]
                    for dc in range(EC):
                        for dh in range(2):
                            nc.tensor.matmul(
                                vps[dh][:, :],
                                XP[:, dc, b * 128 : (b + 1) * 128],
                                WV[:, dc, dh * 512 : (dh + 1) * 512],
                                start=(dc == 0),
                                stop=(dc == EC - 1),
                            )
                    for dh in range(2):
                        nc.vector.tensor_tensor(
                            out=VOWN[:, b, dh * 512 : (dh + 1) * 512],
                            in0=vps[dh][:, :],
                            in1=BVF[:, dh * 512 : (dh + 1) * 512],
                            op=mybir.AluOpType.add,
                        )
                    if b % 2 == 1:
                        bnc = bnc_v1_in if b < 4 else bnc_v2_in
                        nc.gpsimd.dma_start(
                            out=bnc[:, (b - 1) % 4 : (b - 1) % 4 + 2, :],
                            in_=VOWN[:, b - 1 : b + 1, :],
                        )
                    if b == 3:
                        nc.gpsimd.collective_compute(
                            "AllGather",
                            mybir.AluOpType.bypass,
                            replica_groups=GROUPS,
                            ins=[bnc_v1_in.opt()],
                            outs=[bnc_v1_out.opt()],
                        )
                    if b == 7:
                        nc.gpsimd.collective_compute(
                            "AllGather",
                            mybir.AluOpType.bypass,
                            replica_groups=GROUPS,
                            ins=[bnc_v2_in.opt()],
                            outs=[bnc_v2_out.opt()],
                        )

                # QT (x 1/32, +bq/32): overlaps the K exchanges. QT shares
                # KTOWN's slot (tag) - allocation waits for the last ship.
                QT = persist.tile([128, EC, 1024], BF16, tag="QTK", name="QT")
                for sh in range(2):
                    for w in range(2):
                        ec0 = 4 * w
                        pss = [
                            pa_psum.tile(
                                [128, 512], F32, tag="pa", name=f"paq{sh}_{w}_{i}"
                            )
                            for i in range(4)
                        ]
                        for dc in range(EC):
                            for i in range(4):
                                nc.tensor.matmul(
                                    pss[i][:, :],
                                    WQ[:, dc, (ec0 + i) * 128 : (ec0 + i + 1) * 128],
                                    XP[:, dc, sh * 512 : (sh + 1) * 512],
                                    start=(dc == 0),
                                    stop=(dc == EC - 1),
                                )
                        for i in range(4):
                            ec = ec0 + i
                            nc.vector.tensor_scalar(
                                out=QT[:, ec, sh * 512 : (sh + 1) * 512],
                                in0=pss[i][:, :],
                                scalar1=inv,
                                scalar2=BQ[:, ec : ec + 1],
                                op0=mybir.AluOpType.mult,
                                op1=mybir.AluOpType.add,
                            )

                # readbacks: interleave the two parity halves straight into
                # logical order; blocks ascending so narrow slots unblock
                # first. ALL on sync: its only phase-B work (g1 stores) is
                # at the very end, so the semaphore waits block nothing,
                # while scalar stays clear for the exps.
                for th in range(2):
                    for b in range(4):
                        lb = 8 * th + 2 * b
                        nc.sync.dma_start(
                            out=KT[:, :, lb * 128 : (lb + 1) * 128],
                            in_=bnc_k_out[0, :, th, :, b * 128 : (b + 1) * 128],
                        )
                        nc.sync.dma_start(
                            out=KT[:, :, (lb + 1) * 128 : (lb + 2) * 128],
                            in_=bnc_k_out[1, :, th, :, b * 128 : (b + 1) * 128],
                        )
                for vh in range(2):
                    bout = bnc_v1_out if vh == 0 else bnc_v2_out
                    for b in range(4):
                        lb = 8 * vh + 2 * b
                        nc.sync.dma_start(
                            out=V[:, lb, :],
                            in_=bout[0, :, b, :],
                        )
                        nc.sync.dma_start(
                            out=V[:, lb + 1, :],
                            in_=bout[1, :, b, :],
                        )

            # ---- phase B + C: attention + output projection ----
            with (
                tc.tile_pool(name="att", bufs=5) as att_pool,
                tc.tile_pool(name="attT", bufs=2) as attT_pool,
                tc.tile_pool(name="ctx", bufs=1) as ctx_pool,
                tc.tile_pool(name="stat", bufs=1) as stat_pool,
                tc.tile_pool(name="sc_psum", bufs=2, space="PSUM") as sc_psum,
                tc.tile_pool(name="mm_psum", bufs=2, space="PSUM") as mm_psum,
                tc.tile_pool(name="tp_psum", bufs=2, space="PSUM") as tp_psum,
                tc.tile_pool(name="outbuf", bufs=2) as out_pool,
            ):
                CTXT = ctx_pool.tile([128, EC, 1024], BF16, tag="CTXT")
                LSUM = stat_pool.tile([128, 2 * NSLOT], F32, tag="LS")
                LTOT = stat_pool.tile([128, NSLOT], F32, tag="LT")

                def scores_slot(g, j, ATTs):
                    slot = g * 4 + j
                    T = _slot_T(slot)
                    ATT = att_pool.tile([128, S], BF16, tag="att", name=f"att{slot}")
                    ATTs[slot] = ATT

                    nparts = (T + 1023) // 1024
                    parts = []
                    for p in range(nparts):
                        w = min(1024, T - p * 1024)
                        sc = sc_psum.tile([128, 1024], F32, tag="sc", name=f"sc{slot}_{p}")
                        parts.append((sc, w))
                    # ec-outer: one LDWEIGHTS per ec covers the whole row
                    for ec in range(EC):
                        for p, (sc, w) in enumerate(parts):
                            for c0 in range(0, w, 512):
                                cw = min(512, w - c0)
                                a0 = p * 1024 + c0
                                nc.tensor.matmul(
                                    sc[:, c0 : c0 + cw],
                                    QT[:, ec, slot * 128 : (slot + 1) * 128],
                                    KT[:, ec, a0 : a0 + cw],
                                    start=(ec == 0),
                                    stop=(ec == EC - 1),
                                )
                    lsc, lw = parts[-1]
                    nc.vector.tensor_tensor(
                        out=lsc[:, lw - 256 : lw],
                        in0=lsc[:, lw - 256 : lw],
                        in1=MASK[:, slot, :],
                        op=mybir.AluOpType.add,
                    )
                    # no max-subtraction: scores ~ N(0,1) (|s|<~6), so exp()
                    # is overflow-safe; each part's exp fires as soon as that
                    # part's scores land
                    for p, (sc, w) in enumerate(parts):
                        nc.scalar.activation(
                            ATT[:, p * 1024 : p * 1024 + w],
                            sc[:, :w],
                            mybir.ActivationFunctionType.Exp,
                            bias=0.0,
                            scale=1.0,
                            accum_out=LSUM[:, 2 * slot + p : 2 * slot + p + 1],
                        )
                    if nparts == 2:
                        nc.vector.tensor_tensor(
                            out=LTOT[:, slot : slot + 1],
                            in0=LSUM[:, 2 * slot : 2 * slot + 1],
                            in1=LSUM[:, 2 * slot + 1 : 2 * slot + 2],
                            op=mybir.AluOpType.add,
                        )
                        nc.vector.reciprocal(
                            RL[:, slot : slot + 1], LTOT[:, slot : slot + 1]
                        )
                    else:
                        nc.vector.reciprocal(
                            RL[:, slot : slot + 1],
                            LSUM[:, 2 * slot : 2 * slot + 1],
                        )

                def transpose_slot(g, j, ATTs, ATT_T):
                    # PE transpose of the slot's attn row into ATT_T, in
                    # groups of up to 4 key blocks per PSUM tile
                    slot = g * 4 + j
                    T = _slot_T(slot)
                    ATT = ATTs[slot]
                    nb = T // 128
                    for g4 in range(0, nb, 4):
                        n = min(4, nb - g4)
                        tp = tp_psum.tile(
                            [128, 512], BF16, tag="tp", name=f"tp{slot}_{g4}"
                        )
                        for tb in range(n):
                            nc.tensor.transpose(
                                tp[:, tb * 128 : (tb + 1) * 128],
                                ATT[:, (g4 + tb) * 128 : (g4 + tb + 1) * 128],
                                IDENT[:, :],
                            )
                        # alternate drain engines to spread the copies
                        ceng = nc.vector.tensor_copy if (g4 // 4) % 2 == 0 else nc.scalar.copy
                        ceng(
                            ATT_T[:, g4 : g4 + n, j * 128 : (j + 1) * 128],
                            tp[:, 0 : n * 128],
                        )

                def attn_v(g, ATT_T):
                    ntg = _slot_T(g * 4 + 3) // 128
                    for dc in range(EC):
                        ps = mm_psum.tile([128, 512], F32, tag="mm", name=f"av{g}_{dc}")
                        for tcn in range(ntg):
                            jmin = 0
                            for jj in range(4):
                                if 256 * (g * 4 + jj + 1) >= 128 * (tcn + 1):
                                    jmin = jj
                                    break
                            scol = jmin * 128
                            nc.tensor.matmul(
                                ps[:, scol:512],
                                V[:, tcn, dc * 128 : (dc + 1) * 128],
                                ATT_T[:, tcn, scol:512],
                                start=(tcn == 0),
                                stop=(tcn == ntg - 1),
                            )
                        # alternate copy engines so the DVE doesn't become
                        # the serial drain before the final out_projs
                        if dc % 2 == 0:
                            nc.vector.tensor_copy(
                                CTXT[:, dc, g * 512 : (g + 1) * 512], ps[:, :]
                            )
                        else:
                            nc.scalar.copy(
                                CTXT[:, dc, g * 512 : (g + 1) * 512], ps[:, :]
                            )

                def out_proj(slot):
                    OUTS = out_pool.tile([128, D], F32, tag="outs", name=f"outs{slot}")
                    for eh in range(2):
                        ps = mm_psum.tile([128, 512], F32, tag="mm", name=f"op{slot}_{eh}")
                        for dc in range(EC):
                            nc.tensor.matmul(
                                ps[:, :],
                                CTXT[:, dc, slot * 128 : (slot + 1) * 128],
                                WO[:, dc, eh * 512 : (eh + 1) * 512],
                                start=(dc == 0),
                                stop=(dc == EC - 1),
                            )
                        nc.vector.scalar_tensor_tensor(
                            out=OUTS[:, eh * 512 : (eh + 1) * 512],
                            in0=ps[:, :],
                            scalar=RL[:, slot : slot + 1],
                            in1=BOF[:, eh * 512 : (eh + 1) * 512],
                            op0=mybir.AluOpType.mult,
                            op1=mybir.AluOpType.add,
                        )
                        # all stores on sync: it is idle from the V2
                        # readback (~140us) onward, while scalar still runs
                        # transpose/CTXT drains
                        store_eng = nc.sync
                        store_eng.dma_start(
                            out=out_d[
                                slot * 128 : (slot + 1) * 128,
                                eh * 512 : (eh + 1) * 512,
                            ],
                            in_=OUTS[:, eh * 512 : (eh + 1) * 512],
                        )

                ATT_T0 = attT_pool.tile([128, NB, 512], BF16, tag="attT", name="attT0")
                ATT_T1 = attT_pool.tile([128, NB, 512], BF16, tag="attT", name="attT1")
                ATTs = {}
                # narrowest slot first: it needs only the first K readback
                # blocks, smoothing the K-arrival edge; both score groups run
                # before attnV g0 so the V exchange latency stays hidden.
                # Transposes trail the scores by one slot so the PE never
                # waits on an exp; slot7's trail after attnV g0.
                plan = [("s", 0, 0), ("s", 0, 1), ("t", 0, 0), ("s", 0, 2),
                        ("t", 0, 1), ("s", 0, 3), ("t", 0, 2), ("s", 1, 0),
                        ("t", 0, 3), ("s", 1, 1), ("t", 1, 0), ("s", 1, 2),
                        ("t", 1, 1), ("s", 1, 3), ("t", 1, 2)]
                for op, g, j in plan:
                    tgt = ATT_T0 if g == 0 else ATT_T1
                    if op == "s":
                        scores_slot(g, j, ATTs)
                    else:
                        transpose_slot(g, j, ATTs, tgt)
                attn_v(0, ATT_T0)
                transpose_slot(1, 3, ATTs, ATT_T1)
                for j in range(4):
                    out_proj(j)
                attn_v(1, ATT_T1)
                for j in range(4):
                    out_proj(4 + j)

    nc.compile()
    return nc


def _core_blocks(core):
    parity = core % 2  # even core (pair rank 0) -> even blocks
    return [2 * s + parity for s in range(NSLOT)]


def _make_in_maps(x, Wq, bq, Wk, bk, Wv, bv, Wo, bo):
    bf = ml_dtypes.bfloat16

    def wt_layout(W):
        return np.ascontiguousarray(
            W.T.astype(bf).reshape(EC, 128, D).transpose(1, 0, 2)
        )

    def xT_layout(xrows, n):
        return np.ascontiguousarray(
            xrows.T.astype(bf).reshape(EC, 128, n).transpose(1, 0, 2)
        )

    wq_l, wk_l, wv_l, wo_l = (wt_layout(W) for W in (Wq, Wk, Wv, Wo))
    bq_l = np.ascontiguousarray(bq.reshape(EC, 128).T.astype(np.float32))
    bk_l = np.ascontiguousarray(bk.reshape(EC, 128).T.astype(np.float32))
    bv_l = np.ascontiguousarray(bv.reshape(1, D).astype(np.float32))
    bo_l = np.ascontiguousarray(bo.reshape(1, D).astype(np.float32))
    ident = np.eye(128, dtype=bf)

    in_maps = []
    for core in range(8):
        b = core // 2
        blocks = _core_blocks(core)
        xb = np.asarray(x[b], dtype=np.float32)
        # parity-compact x: own blocks (used for Q, K-own, V-own)
        xp = np.concatenate([xb[bl * 128 : (bl + 1) * 128] for bl in blocks], axis=0)
        # tail mask over the last 256 logical key columns of each slot
        mask = np.zeros((128, NSLOT, 256), np.float32)
        r = np.arange(128)[:, None]
        jj = np.arange(256)[None, :]
        for s_i, bl in enumerate(blocks):
            lim = bl * 128 + r
            t_idx = 256 * s_i + jj
            mask[:, s_i, :] = np.where(t_idx <= lim, 0.0, NEG)
        in_maps.append(
            {
                "xpT": xT_layout(xp, 1024),
                "wqT": wq_l,
                "wkT": wk_l,
                "wvT": wv_l,
                "woT": wo_l,
                "bq": bq_l,
                "bk": bk_l,
                "bv": bv_l,
                "bo": bo_l,
                "mask": mask,
                "ident": ident,
            }
        )
    return in_maps


def _run(inputs, trace=False):
    global _compiled
    if _compiled is None:
        _compiled = _build()
    nc = _compiled
    in_maps = _make_in_maps(**inputs)
    res = run_bass_kernel_spmd(nc, in_maps, core_ids=list(range(8)), trace=trace)
    out = np.zeros((B, S, D), np.float32)
    for core in range(8):
        b = core // 2
        o = res.results[core]["out"]
        for s_i, bl in enumerate(_core_blocks(core)):
            out[b, bl * 128 : (bl + 1) * 128, :] = o[s_i * 128 : (s_i + 1) * 128, :]
    return out, res


def kernel(**inputs):
    out, _ = _run(inputs, trace=False)
    return out
